# revision 3
# baseline (speedup 1.0000x reference)
"""Trainium2 Bass kernel for DecouplePreAggGraphConv (GNN message passing).

out[b,j,:] = diag(adj)[j] * (x[b,j] @ W0[j])
           + sum_k offdiag(adj)[j,k] * (x[b,k] @ W1[k])
           + bias

Data-parallel over B across 8 NeuronCores. Host pre-transposes x to
[tile, FIN, J, TB] bf16 so the per-joint stationaries are direct SBUF
slices (no on-chip transposes). Per 128-row tile:

  1. one DMA load of xT tile [128, J*TB] bf16
  2. per-joint GEMM  h_k = xT_k.T @ [diag_k*W0_k | W1_k] -> PSUM [128,256]
  3. drain h to SBUF bf16 (h0 half gets +bias), giving h_sb [b,(h,k,m)]
  4. bounce through a DRAM scratch to move k onto partitions:
     2 scatter DMAs (b-halves i=0,1) + 1 contiguous readback into
     hp_sb [(i,h,k)=68, (g,m)=8192]  (b = i*64 + g)
  5. mixing GEMM with constant [68,34] stationary (identity block for the
     self term, off.T blocks for the neighbor mix) -> PSUM [34, 512] x16
  6. drain to o_sb bf16 [34, 8192]; one contiguous store to out_dev.

Host un-shuffles out_dev [nt,34,8192] -> [bs,J,FOUT] f32. All HBM traffic
is bf16: x 0.56 + bounce 2.23 + out 0.56 = 3.3 MB/tile.
"""

import os
import sys

for _p in ("/opt/trn_rl_repo", "/root/.axon_site/_ro/trn_rl_repo"):
    if os.path.isdir(_p) and _p not in sys.path:
        sys.path.insert(0, _p)

import numpy as np

import concourse.bass as bass
import concourse.mybir as mybir
import concourse.tile as tile
from concourse import bacc
from concourse import bass_utils as _bu
from concourse.bass_utils import run_bass_kernel_spmd

B, J, FIN, FOUT = 16384, 17, 128, 128
N_CORES = 8
TB = 128            # batch rows per tile
NI = 2              # partition groups (b = i*64 + g)
NG = TB // NI       # 64 batch rows per group
HROWS = NI * 2 * J  # 68 rows of the shuffled h tile (i, h, k)
OROWS = NI * J      # 34 rows of the mix output (i, j)
HPF = NG * FOUT     # 8192 free size of the shuffled tile
MIXCH = 512         # mix psum chunk (free elems, one 2KB bank)
F32 = mybir.dt.float32
BF16 = mybir.dt.bfloat16

_prog_cache: dict[tuple, object] = {}


def _build_program(bs: int, repeat: int = 1):
    """Build the SPMD Bass program for a per-core batch shard of `bs` rows."""
    nt = bs // TB
    assert bs % TB == 0

    nc = bacc.Bacc("TRN2", target_bir_lowering=False, debug=False,
                   num_devices=N_CORES)

    xs = nc.declare_dram_parameter("xs", [nt, FIN, J, TB], BF16,
                                   isOutput=False)
    wcat = nc.declare_dram_parameter("wcat", [FIN, J, 2 * FOUT], BF16,
                                     isOutput=False)
    mix2 = nc.declare_dram_parameter("mix2", [HROWS, OROWS], BF16,
                                     isOutput=False)
    biasb = nc.declare_dram_parameter("biasb", [TB, FOUT], F32,
                                      isOutput=False)
    out = nc.declare_dram_parameter("out", [nt, OROWS, HPF], BF16,
                                    isOutput=True)

    with tile.TileContext(nc) as tc:
        with (
            tc.tile_pool(name="const", bufs=1) as cpool,
            tc.tile_pool(name="x", bufs=2) as xpool,
            tc.tile_pool(name="hsb", bufs=2) as hpool,
            tc.tile_pool(name="hp", bufs=2) as hppool,
            tc.tile_pool(name="osb", bufs=2) as opool,
            tc.tile_pool(name="s1p", bufs=2, space=bass.MemorySpace.PSUM) as s1p,
            tc.tile_pool(name="mxp", bufs=2, space=bass.MemorySpace.PSUM) as mxp,
        ):
            # ---- constants, loaded once ----
            wcat_sb = cpool.tile([FIN, J, 2 * FOUT], BF16, tag="wcat")
            nc.sync.dma_start(wcat_sb[:], wcat[:])
            mix2_sb = cpool.tile([HROWS, OROWS], BF16, tag="mix2")
            nc.sync.dma_start(mix2_sb[:], mix2[:])
            biasb_sb = cpool.tile([TB, FOUT], F32, tag="biasb")
            nc.sync.dma_start(biasb_sb[:], biasb[:])

            # ping-pong DRAM scratch for the reshuffle bounce
            scrs = [nc.dram_tensor(f"scr{p}", [HROWS, HPF], BF16)
                    for p in range(2)]

            for t in range(nt * repeat):
                t = t % nt
                # 1. load pre-transposed x tile [n, (k, b)]
                xt = xpool.tile([FIN, J, TB], BF16, tag="x")
                nc.sync.dma_start(xt[:], xs[t])

                # 2/3. per joint: GEMM, drain (h0 half + bias, cast bf16)
                h_sb = hpool.tile([TB, 2, J, FOUT], BF16, tag="h")
                for k in range(J):
                    hk = s1p.tile([TB, 2 * FOUT], F32, tag="hk")
                    nc.tensor.matmul(hk[:], xt[:, k, :], wcat_sb[:, k, :])
                    nc.vector.tensor_add(h_sb[:, 0, k, :], hk[:, :FOUT],
                                         biasb_sb[:])
                    nc.scalar.copy(h_sb[:, 1, k, :], hk[:, FOUT:])

                # 4. bounce: scatter b-halves into DRAM scratch laid out as
                # [(i,h,k), (g,m)], then read back contiguously.
                scr = scrs[t % 2]
                sv = scr.rearrange("(i h k) (g m) -> i g h k m",
                                   i=NI, h=2, k=J, g=NG, m=FOUT)
                for i in range(NI):
                    nc.scalar.dma_start(sv[i], h_sb[i * NG:(i + 1) * NG])
                hp = hppool.tile([HROWS, HPF], BF16, tag="hp")
                nc.sync.dma_start(hp[:], scr[:])

                # 5. mix GEMM chunks, drain bf16
                o_sb = opool.tile([OROWS, HPF], BF16, tag="osb")
                for c in range(HPF // MIXCH):
                    f0 = c * MIXCH
                    mp = mxp.tile([OROWS, MIXCH], F32, tag="mx")
                    nc.tensor.matmul(mp[:], mix2_sb[:],
                                     hp[:, f0:f0 + MIXCH])
                    if c % 2 == 0:
                        nc.vector.tensor_copy(o_sb[:, f0:f0 + MIXCH], mp[:])
                    else:
                        nc.scalar.copy(o_sb[:, f0:f0 + MIXCH], mp[:])

                # 6. one contiguous store
                nc.sync.dma_start(out[t], o_sb[:])

    nc.compile()
    return nc


def _host_prep(x, W, bias, adj, bs):
    """Build the per-core input maps (layouts described in the docstring)."""
    import ml_dtypes
    nt = bs // TB
    diag = np.diagonal(adj).astype(np.float32)
    off = (adj * (1.0 - np.eye(J, dtype=adj.dtype))).astype(np.float32)

    # stage-1 weights: [FIN, J, 2*FOUT], columns = [diag_k*W0_k | W1_k]
    wcat = np.concatenate([diag[:, None, None] * W[0], W[1]], axis=2)
    wcat = np.ascontiguousarray(wcat.transpose(1, 0, 2))

    # mix stationary [HROWS=(i,h,k), OROWS=(i,j)]
    m2 = np.zeros((NI, 2, J, NI, J), dtype=np.float32)
    for i in range(NI):
        m2[i, 0, :, i, :] = np.eye(J, dtype=np.float32)   # self (h0s) rows
        m2[i, 1, :, i, :] = off.T                         # h1 rows
    mix2 = m2.reshape(HROWS, OROWS)

    biasb = np.broadcast_to(bias.astype(np.float32), (TB, FOUT))

    shared = {
        "wcat": wcat.astype(ml_dtypes.bfloat16),
        "mix2": mix2.astype(ml_dtypes.bfloat16),
        "biasb": np.ascontiguousarray(biasb),
    }
    in_maps = []
    for c in range(N_CORES):
        m = dict(shared)
        # [bs,J,FIN] -> [nt, FIN, J, TB] bf16 (pre-transposed per tile)
        xc = x[c * bs:(c + 1) * bs].reshape(nt, TB, J, FIN)
        m["xs"] = np.ascontiguousarray(
            xc.transpose(0, 3, 2, 1)).astype(ml_dtypes.bfloat16)
        in_maps.append(m)
    return in_maps


def _unshuffle_out(res_out, bs):
    """[nt, OROWS, HPF] bf16 -> [bs, J, FOUT] f32 (b = t*TB + i*NG + g)."""
    nt = bs // TB
    o = np.asarray(res_out).reshape(nt, NI, J, NG, FOUT)
    return np.ascontiguousarray(
        o.transpose(0, 1, 3, 2, 4)).reshape(bs, J, FOUT).astype(np.float32)


def _run(x, W, bias, adj, bs, profile=False, tmpdir=None):
    key = (bs,)
    if key not in _prog_cache:
        _prog_cache[key] = _build_program(bs)
    nc = _prog_cache[key]
    in_maps = _host_prep(x, W, bias, adj, bs)
    res = run_bass_kernel_spmd(nc, in_maps, list(range(N_CORES)),
                               trace=profile, tmpdir=tmpdir)
    out = np.concatenate(
        [_unshuffle_out(res.results[c]["out"], bs) for c in range(N_CORES)],
        axis=0)
    if profile:
        return out, res
    return out


def kernel(x, W, bias, adj):
    x = np.asarray(x, dtype=np.float32)
    W = np.asarray(W, dtype=np.float32)
    bias = np.asarray(bias, dtype=np.float32)
    adj = np.asarray(adj, dtype=np.float32)
    assert x.shape == (B, J, FIN)
    return _run(x, W, bias, adj, B // N_CORES)


# revision 11
# speedup vs baseline: 1.3227x; 1.3227x over previous
"""Trainium2 Bass kernel for DecouplePreAggGraphConv (GNN message passing).

out[b,j,:] = diag(adj)[j] * (x[b,j] @ W0[j])
           + sum_k offdiag(adj)[j,k] * (x[b,k] @ W1[k])
           + bias

Decoupled two-GEMM formulation, data-parallel over B across 8 NeuronCores,
all HBM traffic bf16. Host pre-transposes x so the 17 per-joint stage-1
stationaries are direct SBUF slices. Per macro-tile (2 x 128 batch rows):

  1. one x-load DMA [128, (s,k,b)]                               (SP)
  2. per subtile s: per-joint GEMM h_k = xT_k.T @ [diag_k*W0_k | W1_k],
     4 joints per [128,1024] PSUM tile, plain-copy drains (bf16 cast)
     alternating DVE/ACT -> h_sb [b, (s,k,h,m)]
  3. 2 scatter DMAs (b-halves i=0,1, both subtiles each) into a DRAM
     scratch laid out [(i,k,h)=68 + bias row, (s,g,m)]  (b = i*64+g);
     row 68 = bias (written once per scratch buffer)   (GPSIMD, DVE)
  4. one readback DMA -> hp [69, (s,g,m)] (k on partitions)      (SP)
  5. mixing GEMM, constant [69,34] stationary (identity block = self
     term, off.T blocks = neighbor mix, ones row = bias): chunk pairs
     packed at PSUM partition offsets 0 and 64 -> one drain per pair
  6. one store DMA of o_sb rows 0:98 (34:64 garbage)             (ACT)

Host un-shuffles out_dev [ntm,98,(s,c,512)] -> [bs,J,FOUT] f32.
"""

import os
import sys

for _p in ("/opt/trn_rl_repo", "/root/.axon_site/_ro/trn_rl_repo"):
    if os.path.isdir(_p) and _p not in sys.path:
        sys.path.insert(0, _p)

import numpy as np

import concourse.bass as bass
import concourse.mybir as mybir
import concourse.tile as tile
from concourse import bacc
from concourse.bass_utils import run_bass_kernel_spmd

B, J, FIN, FOUT = 16384, 17, 128, 128
N_CORES = 8
TB = 128            # batch rows per subtile
TS = 2              # subtiles per macro-tile
NI = 2              # partition groups (b = i*64 + g)
NG = TB // NI       # 64 batch rows per group
HROWS = NI * J * 2  # 68 data rows of the shuffled h tile (i, k, h)
OROWS = NI * J      # 34 rows of one mix chunk (i, j)
HPF = NG * FOUT     # 8192 free size of the shuffled tile per subtile
MIXCH = 512         # mix psum chunk (free elems, one 2KB bank)
NCH = HPF // MIXCH  # 16 chunks per subtile (8 pairs)
ODD = 64            # partition offset of odd chunks in the mix psum tile
SROWS = ODD + OROWS  # 98 rows stored (34:64 garbage, dropped on host)
KG = 5              # stage-1 psum groups: k 0-3,4-7,8-11,12-15,16
F32 = mybir.dt.float32
BF16 = mybir.dt.bfloat16

_prog_cache: dict[tuple, object] = {}


def _build_program(bs: int, repeat: int = 1):
    """Build the SPMD Bass program for a per-core batch shard of `bs` rows."""
    ntm = bs // (TB * TS)
    assert bs % (TB * TS) == 0

    nc = bacc.Bacc("TRN2", target_bir_lowering=False, debug=False,
                   num_devices=N_CORES)

    xs = nc.declare_dram_parameter("xs", [ntm, FIN, TS, J, TB], BF16,
                                   isOutput=False)
    wcat = nc.declare_dram_parameter("wcat", [FIN, J, 2 * FOUT], BF16,
                                     isOutput=False)
    mix2 = nc.declare_dram_parameter("mix2", [HROWS + 1, OROWS], BF16,
                                     isOutput=False)
    brow = nc.declare_dram_parameter("brow", [TS, NG, FOUT], BF16,
                                     isOutput=False)
    out = nc.declare_dram_parameter("out", [ntm, SROWS, TS, NCH // 2, MIXCH],
                                    BF16, isOutput=True)

    with tile.TileContext(nc) as tc:
        with (
            tc.tile_pool(name="const", bufs=1) as cpool,
            tc.tile_pool(name="x", bufs=2) as xpool,
            tc.tile_pool(name="hsb", bufs=2) as hpool,
            tc.tile_pool(name="hp", bufs=2) as hppool,
            tc.tile_pool(name="osb", bufs=2) as opool,
            tc.tile_pool(name="s1p", bufs=2, space=bass.MemorySpace.PSUM) as s1p,
            tc.tile_pool(name="mxp", bufs=2, space=bass.MemorySpace.PSUM) as mxp,
        ):
            # ---- constants, loaded once ----
            wcat_sb = cpool.tile([FIN, J, 2 * FOUT], BF16, tag="wcat")
            nc.sync.dma_start(wcat_sb[:], wcat[:])
            mix2_sb = cpool.tile([HROWS + 1, OROWS], BF16, tag="mix2")
            nc.sync.dma_start(mix2_sb[:], mix2[:])

            # ping-pong DRAM scratch; row 68 = bias, written once
            scrs = [nc.dram_tensor(f"scr{p}", [HROWS + 1, TS, NG, FOUT], BF16)
                    for p in range(2)]
            for p in range(2):
                nc.sync.dma_start(scrs[p][HROWS], brow[:])

            for t in range(ntm * repeat):
                t = t % ntm
                xt = xpool.tile([FIN, TS, J, TB], BF16, tag="x")
                nc.sync.dma_start(xt[:], xs[t])

                h_sb = hpool.tile([TB, TS, J, 2, FOUT], BF16, tag="h")
                for s in range(TS):
                    for g4 in range(KG):
                        k0 = g4 * 4
                        kw = min(4, J - k0)
                        ps = s1p.tile([TB, 4 * 2 * FOUT], F32, tag="hk")
                        for kk in range(kw):
                            nc.tensor.matmul(
                                ps[:, kk * 256:(kk + 1) * 256],
                                xt[:, s, k0 + kk, :],
                                wcat_sb[:, k0 + kk, :])
                        nc.vector.tensor_copy(
                            h_sb[:, s, k0:k0 + kw, :, :], ps[:, :kw * 256])

                # scatter b-halves, one DMA per (i, subtile)
                scr = scrs[t % 2]
                sv = scr[:HROWS].rearrange("(i k h) s g m -> i s g k h m",
                                           i=NI, k=J, h=2)
                for s in range(TS):
                    nc.gpsimd.dma_start(sv[0, s], h_sb[0:NG, s])
                    nc.scalar.dma_start(sv[1, s], h_sb[NG:TB, s])

                # readback: k on partitions (+ bias row)
                hp = hppool.tile([HROWS + 1, TS, NG * FOUT], BF16, tag="hp")
                nc.sync.dma_start(hp[:], scr[:])

                # mix GEMM chunk pairs at psum partition offsets 0 / ODD
                o_sb = opool.tile([TB, TS, NCH // 2, MIXCH], BF16, tag="osb")
                for s in range(TS):
                    for c in range(NCH // 2):
                        mp = mxp.tile([TB, MIXCH], F32, tag="mx")
                        f0 = 2 * c * MIXCH
                        nc.tensor.matmul(mp[0:OROWS, :], mix2_sb[:],
                                         hp[:, s, f0:f0 + MIXCH])
                        nc.tensor.matmul(mp[ODD:SROWS, :], mix2_sb[:],
                                         hp[:, s, f0 + MIXCH:f0 + 2 * MIXCH])
                        if c % 4 == 3:
                            nc.scalar.copy(o_sb[:, s, c, :], mp[:])
                        else:
                            nc.vector.tensor_copy(o_sb[:, s, c, :], mp[:])

                nc.gpsimd.dma_start(out[t], o_sb[0:SROWS])

    nc.compile()
    return nc


def _host_prep(x, W, bias, adj, bs):
    """Build the per-core input maps (layouts described in the docstring)."""
    import ml_dtypes
    ntm = bs // (TB * TS)
    diag = np.diagonal(adj).astype(np.float32)
    off = (adj * (1.0 - np.eye(J, dtype=adj.dtype))).astype(np.float32)

    # stage-1 weights: [FIN, J, 2*FOUT], columns = [diag_k*W0_k | W1_k]
    wcat = np.concatenate([diag[:, None, None] * W[0], W[1]], axis=2)
    wcat = np.ascontiguousarray(wcat.transpose(1, 0, 2))

    # mix stationary [(i,k,h) + bias row, (i,j)]
    m2 = np.zeros((NI, J, 2, NI, J), dtype=np.float32)
    for i in range(NI):
        m2[i, :, 0, i, :] = np.eye(J, dtype=np.float32)   # self (h0s) rows
        m2[i, :, 1, i, :] = off.T                         # h1 rows
    mix2 = np.concatenate(
        [m2.reshape(HROWS, OROWS), np.ones((1, OROWS), np.float32)], axis=0)

    brow = np.broadcast_to(bias.astype(np.float32), (TS, NG, FOUT))

    shared = {
        "wcat": wcat.astype(ml_dtypes.bfloat16),
        "mix2": mix2.astype(ml_dtypes.bfloat16),
        "brow": np.ascontiguousarray(brow).astype(ml_dtypes.bfloat16),
    }
    in_maps = []
    for c in range(N_CORES):
        m = dict(shared)
        # [bs,J,FIN] -> [ntm, FIN, TS, J, TB] bf16 (pre-transposed)
        xc = x[c * bs:(c + 1) * bs].reshape(ntm, TS, TB, J, FIN)
        m["xs"] = np.ascontiguousarray(
            xc.transpose(0, 4, 1, 3, 2)).astype(ml_dtypes.bfloat16)
        in_maps.append(m)
    return in_maps


def _unshuffle_out(res_out, bs):
    """[ntm, SROWS, TS, NCH//2, MIXCH] bf16 -> [bs, J, FOUT] f32.

    row r<34: chunk parity 0; row 64+r: parity 1 (34:64 garbage).
    chunk q = 2*c+p covers g = 4*q + gg; b = (t*TS+s)*TB + i*NG + g.
    """
    ntm = bs // (TB * TS)
    o = np.asarray(res_out).reshape(ntm, SROWS, TS, NCH // 2, 4, FOUT)
    res = np.empty((ntm, TS, NI, NG, J, FOUT), dtype=np.float32)
    for p in range(2):
        rows = o[:, p * ODD:p * ODD + OROWS]          # [ntm,34,TS,8,4,F]
        rows = rows.reshape(ntm, NI, J, TS, NCH // 2, 4, FOUT)
        gidx = (2 * np.arange(NCH // 2)[:, None] + p) * 4 + np.arange(4)
        res[:, :, :, gidx.reshape(-1)] = (
            rows.transpose(0, 3, 1, 4, 5, 2, 6).reshape(
                ntm, TS, NI, NCH // 2 * 4, J, FOUT))
    return res.reshape(bs, J, FOUT)


def _run(x, W, bias, adj, bs, profile=False, tmpdir=None):
    key = (bs,)
    if key not in _prog_cache:
        _prog_cache[key] = _build_program(bs)
    nc = _prog_cache[key]
    in_maps = _host_prep(x, W, bias, adj, bs)
    res = run_bass_kernel_spmd(nc, in_maps, list(range(N_CORES)),
                               trace=profile, tmpdir=tmpdir)
    out = np.concatenate(
        [_unshuffle_out(res.results[c]["out"], bs) for c in range(N_CORES)],
        axis=0)
    if profile:
        return out, res
    return out


def kernel(x, W, bias, adj):
    x = np.asarray(x, dtype=np.float32)
    W = np.asarray(W, dtype=np.float32)
    bias = np.asarray(bias, dtype=np.float32)
    adj = np.asarray(adj, dtype=np.float32)
    assert x.shape == (B, J, FIN)
    return _run(x, W, bias, adj, B // N_CORES)
